# revision 12
# baseline (speedup 1.0000x reference)
"""APNB block (sparse pyramid attention) distributed over 8 TRN2 NeuronCores.

Sharding: core c = 2*b + h handles batch item b (of 4) and row-half h (of 2,
60 rows each).  All three 1x1 convs and the attention are data-parallel over
the 7200 local pixels.  The only cross-core data are the pyramid-pooled
key/value sums: p6 (20x20) and p8 (15x15) pooling blocks split cleanly at the
row-60 boundary, so each pair of cores AllGathers its half-grids (p1/p3 are
derived from the p6 grid afterwards).

Layouts (per core):
  kq  [ck=256, n=7200]   channel-major, SBUF-resident (query & key source)
  v   [cv=256, n]        transient per row-group, pooled from PSUM
  scoresT [s=110, n]     pixels on the free axis -> full-rate f32r matmuls
  value   [s=110, cv]    region-major (transposed once after AllGather)
  agg     [cv, n], out [co, n]  -> contiguous output DMA
All matmuls run as float32r (full PE rate at free-dim >= 256, data stays f32).
"""

import sys

sys.path.insert(0, "/opt/trn_rl_repo")

import numpy as np

import concourse.bass as bass
import concourse.mybir as mybir
import concourse.tile as tile
from concourse.vector_clock import ScopedClock

F32 = mybir.dt.float32
F32R = mybir.dt.float32r
AF = mybir.ActivationFunctionType
ALU = mybir.AluOpType

N_CORES = 8
B, CIN, H, W = 4, 512, 120, 120
CK, CV, COUT = 256, 256, 512
HL = H // 2          # 60 local rows per core
NPIX = HL * W        # 7200 local pixels
S = 110              # 1 + 9 + 36 + 64 pooled regions
RG = 4               # rows per conv group
NGRP = HL // RG      # 15 conv groups
PIXT = 512           # attention pixel tile
EPS = 1e-5
# per-half pooled partial grid: p6 3x6=18 + p8 4x8=32 regions
HALF_GRID = 50


def _r(ap):
    return ap.bitcast(F32R)


class SplitDrainTC(tile.TileContext):
    """TileContext whose kernel-tail drain splits its semaphore waits into
    one wait instruction per semaphore (walrus rejects >4 waits on one
    instruction, and the tail drain otherwise aggregates every live proc)."""

    def _drain_and_barrier(self, tick_clock, wait_clock):
        nc = self.nc
        nc.sync.drain()
        probe = mybir.InstNoOp(
            name="wait-probe", ins=[], outs=[], engine=mybir.EngineType.SP
        )
        wait_clock.add_sem_waits(probe, ScopedClock({None: tick_clock.global_clock}))
        waits = list(probe.sync_info.on_wait or []) if probe.sync_info else []
        name2handle = {
            getattr(h, "name", None): h for h in wait_clock.sems.allocated().values()
        }
        for w in waits:
            h = name2handle.get(w.ant_name)
            assert h is not None, f"no sem handle for {w.ant_name}"
            nc.sync.wait_ge(h, w.wait_value)
        nc.all_engine_barrier()
        popped = nc._tile_sem_poison_stack.pop()
        assert popped is self._sem_poison
        nc.clear_and_free_semaphores(list(self.sems.allocated().values()))
        nc.all_engine_barrier()


def _wait_limit(ins):
    # walrus setupSyncWait capacity differs per ISA struct (S3_LW matmul and
    # S3D3_AC activation both reject >1); one wait per instruction is the
    # only budget observed safe everywhere.
    return 1


def _split_excess_waits(nc):
    """Walrus codegen rejects instructions with too many sync waits; move the
    excess onto EventSemaphore instructions just before the owner (same
    engine queue, so the ordering semantics are identical)."""
    for bb in nc.main_func.blocks:
        il = list(bb.instructions)
        out = []
        changed = False
        for ins in il:
            limit = _wait_limit(ins)
            si = ins.sync_info
            waits = list(si.on_wait) if si is not None and si.on_wait else []
            if len(waits) > limit:
                changed = True
                pre, keep = waits[:-limit], waits[-limit:]
                for j in range(0, len(pre), 1):
                    ev = mybir.InstEventSemaphore(
                        name=f"wsplit-{ins.name}-{j}",
                        ins=[],
                        outs=[],
                        engine=ins.engine,
                        sync_info=mybir.SyncInfo(
                            on_wait=pre[j : j + 1], on_update=[]
                        ),
                    )
                    nc.register_instruction(ev)
                    out.append(ev)
                ins.sync_info = mybir.SyncInfo(
                    on_wait=keep, on_update=list(si.on_update or [])
                )
            out.append(ins)
        if changed:
            bb.instructions = out


def build_nc():
    nc = bass.Bass(num_devices=N_CORES)

    x_sh = nc.declare_dram_parameter("x_sh", [CIN, HL, W], F32, isOutput=False)
    wkT = nc.declare_dram_parameter("wkT", [CIN, CK], F32, isOutput=False)
    wvT = nc.declare_dram_parameter("wvT", [CIN, CV], F32, isOutput=False)
    woT = nc.declare_dram_parameter("woT", [CV, COUT], F32, isOutput=False)
    bn_inv = nc.declare_dram_parameter("bn_inv", [CK, 1], F32, isOutput=False)
    bn_shift = nc.declare_dram_parameter("bn_shift", [CK, 1], F32, isOutput=False)
    bo = nc.declare_dram_parameter("bo", [COUT, 1], F32, isOutput=False)
    bv_bcast = nc.declare_dram_parameter("bv_bcast", [S, CV], F32, isOutput=False)
    key_crecip = nc.declare_dram_parameter("key_crecip", [128, S], F32, isOutput=False)
    val_crecip = nc.declare_dram_parameter("val_crecip", [S, 1], F32, isOutput=False)
    ones_mat = nc.declare_dram_parameter("ones_mat", [S, S], F32, isOutput=False)
    ident = nc.declare_dram_parameter("ident", [128, 128], F32, isOutput=False)
    out_d = nc.declare_dram_parameter("out", [COUT, NPIX], F32, isOutput=True)

    with nc.allow_low_precision("fp32r matmul operand rounding"), SplitDrainTC(nc) as tc:
        with (
            tc.tile_pool(name="persist", bufs=1) as persist,
            tc.tile_pool(name="dram", bufs=1, space="DRAM") as dram,
        ):
            # ---- constants to SBUF ----
            wk_sb = []
            wv_sb = []
            for ki in range(4):
                t = persist.tile([128, CK], F32, tag=f"wk{ki}", name=f"wk{ki}")
                nc.sync.dma_start(out=_r(t), in_=_r(wkT[ki * 128 : (ki + 1) * 128, :]))
                wk_sb.append(t)
                t = persist.tile([128, CV], F32, tag=f"wv{ki}", name=f"wv{ki}")
                nc.sync.dma_start(out=_r(t), in_=_r(wvT[ki * 128 : (ki + 1) * 128, :]))
                wv_sb.append(t)
            wo_sb = []
            for mj in range(2):
                t = persist.tile([128, COUT], F32, tag=f"wo{mj}", name=f"wo{mj}")
                nc.sync.dma_start(out=_r(t), in_=_r(woT[mj * 128 : (mj + 1) * 128, :]))
                wo_sb.append(t)
            bn_inv_sb, bn_shift_sb = [], []
            for mj in range(2):
                t = persist.tile([128, 1], F32, tag=f"bninv{mj}", name=f"bninv{mj}")
                nc.sync.dma_start(out=t, in_=bn_inv[mj * 128 : (mj + 1) * 128, :])
                bn_inv_sb.append(t)
                t = persist.tile([128, 1], F32, tag=f"bnshift{mj}", name=f"bnshift{mj}")
                nc.sync.dma_start(out=t, in_=bn_shift[mj * 128 : (mj + 1) * 128, :])
                bn_shift_sb.append(t)
            bo_sb = []
            for co in range(4):
                t = persist.tile([128, 1], F32, tag=f"bo{co}", name=f"bo{co}")
                nc.sync.dma_start(out=t, in_=bo[co * 128 : (co + 1) * 128, :])
                bo_sb.append(t)
            bv_sb = persist.tile([S, CV], F32, tag="bv", name="bv")
            nc.sync.dma_start(out=bv_sb, in_=bv_bcast[:, :])
            kcr_sb = persist.tile([128, S], F32, tag="kcr", name="kcr")
            nc.sync.dma_start(out=kcr_sb, in_=key_crecip[:, :])
            vcr_sb = persist.tile([S, 1], F32, tag="vcr", name="vcr")
            nc.sync.dma_start(out=vcr_sb, in_=val_crecip[:, :])
            ones_sb = persist.tile([S, S], F32, tag="ones", name="ones")
            nc.sync.dma_start(out=_r(ones_sb), in_=_r(ones_mat[:, :]))
            ident_sb = persist.tile([128, 128], F32, tag="ident", name="ident")
            nc.sync.dma_start(out=ident_sb, in_=ident[:, :])

            # ---- persistent working buffers ----
            kq_sb = [persist.tile([128, NPIX], F32, tag=f"kq{mj}", name=f"kq{mj}") for mj in range(2)]
            kq_rs5 = [
                persist.tile([128, HL, 24], F32, tag=f"kqrs{mj}", name=f"kqrs{mj}") for mj in range(2)
            ]
            v_rs5 = [
                persist.tile([128, HL, 24], F32, tag=f"vrs{mj}", name=f"vrs{mj}") for mj in range(2)
            ]
            kqp_sb = [
                persist.tile([128, HALF_GRID], F32, tag=f"kqp{mj}", name=f"kqp{mj}") for mj in range(2)
            ]
            vp_sb = [
                persist.tile([128, HALF_GRID], F32, tag=f"vp{mj}", name=f"vp{mj}") for mj in range(2)
            ]
            key_sb = [persist.tile([128, S], F32, tag=f"key{mj}", name=f"key{mj}") for mj in range(2)]
            vch_sb = [persist.tile([128, S], F32, tag=f"vch{mj}", name=f"vch{mj}") for mj in range(2)]
            value_sb = persist.tile([S, CV], F32, tag="value", name="value")

            ag_in = dram.tile([4 * 128, HALF_GRID], F32)
            ag_out = dram.tile([8 * 128, HALF_GRID], F32)

            # ================= Phase A: convs + pooled partial sums ========
            with (
                tc.tile_pool(name="xin", bufs=2) as xin,
                tc.tile_pool(name="ps_kq", bufs=2, space="PSUM") as ps_kq,
                tc.tile_pool(name="ps_v", bufs=2, space="PSUM") as ps_v,
            ):
                for g in range(NGRP):
                    xt = []
                    for ki in range(4):
                        t = xin.tile([128, RG * W], F32, tag=f"x{ki}", name=f"x{ki}")
                        nc.sync.dma_start(
                            out=_r(t.rearrange("p (a b) -> p a b", a=RG)),
                            in_=_r(x_sh[ki * 128 : (ki + 1) * 128, g * RG : (g + 1) * RG, :]),
                        )
                        xt.append(t)
                    sl = slice(g * RG * W, (g + 1) * RG * W)
                    for mj in range(2):
                        pk = ps_kq.tile([128, RG * W], F32, tag="pkq", name="pkq")
                        for ki in range(4):
                            nc.tensor.matmul(
                                pk,
                                _r(wk_sb[ki][:, mj * 128 : (mj + 1) * 128]),
                                _r(xt[ki]),
                                start=(ki == 0),
                                stop=(ki == 3),
                            )
                        nc.scalar.activation(
                            _r(kq_sb[mj][:, sl]),
                            pk,
                            AF.Relu,
                            bias=bn_shift_sb[mj],
                            scale=bn_inv_sb[mj],
                        )
                        nc.vector.tensor_reduce(
                            kq_rs5[mj][:, g * RG : (g + 1) * RG, :],
                            kq_sb[mj][:, sl].rearrange(
                                "p (r c f) -> p r c f", r=RG, c=24
                            ),
                            axis=mybir.AxisListType.X,
                            op=ALU.add,
                        )
                    for mj in range(2):
                        pv = ps_v.tile([128, RG * W], F32, tag="pv", name="pv")
                        for ki in range(4):
                            nc.tensor.matmul(
                                pv,
                                _r(wv_sb[ki][:, mj * 128 : (mj + 1) * 128]),
                                _r(xt[ki]),
                                start=(ki == 0),
                                stop=(ki == 3),
                            )
                        nc.vector.tensor_reduce(
                            v_rs5[mj][:, g * RG : (g + 1) * RG, :],
                            pv.rearrange("p (r c f) -> p r c f", r=RG, c=24),
                            axis=mybir.AxisListType.X,
                            op=ALU.add,
                        )

                # ---- combine row sums into the p6/p8 half grids ----
                with tc.tile_pool(name="combine", bufs=2) as comb:
                    for src, dst in ((kq_rs5, kqp_sb), (v_rs5, vp_sb)):
                        for mj in range(2):
                            t6 = comb.tile([128, HL, 6], F32, tag="t6", name="t6")
                            nc.vector.tensor_reduce(
                                t6,
                                src[mj].rearrange("p r (j f) -> p r j f", f=4),
                                axis=mybir.AxisListType.X,
                                op=ALU.add,
                            )
                            nc.vector.tensor_reduce(
                                dst[mj][:, 0:18].rearrange("p (a b) -> p a b", a=3),
                                t6.rearrange("p (rb rr) j -> p rb j rr", rr=20),
                                axis=mybir.AxisListType.X,
                                op=ALU.add,
                            )
                            t8 = comb.tile([128, HL, 8], F32, tag="t8", name="t8")
                            nc.vector.tensor_reduce(
                                t8,
                                src[mj].rearrange("p r (j f) -> p r j f", f=3),
                                axis=mybir.AxisListType.X,
                                op=ALU.add,
                            )
                            nc.vector.tensor_reduce(
                                dst[mj][:, 18:50].rearrange("p (a b) -> p a b", a=4),
                                t8.rearrange("p (rb rr) j -> p rb j rr", rr=15),
                                axis=mybir.AxisListType.X,
                                op=ALU.add,
                            )

            # ================= Phase B: AllGather + key/value fixup ========
            for mj in range(2):
                nc.sync.dma_start(out=ag_in[mj * 128 : (mj + 1) * 128, :], in_=kqp_sb[mj])
                nc.sync.dma_start(
                    out=ag_in[(2 + mj) * 128 : (3 + mj) * 128, :], in_=vp_sb[mj]
                )
            nc.gpsimd.collective_compute(
                "AllGather",
                ALU.bypass,
                replica_groups=[[0, 1], [2, 3], [4, 5], [6, 7]],
                ins=[ag_in[:, :].opt()],
                outs=[ag_out[:, :].opt()],
            )
            # ag_out rows: [h0: kq0 kq1 v0 v1][h1: kq0 kq1 v0 v1] (128 each)
            for mj in range(2):
                for tix, dst, cast in ((mj, key_sb[mj], True), (2 + mj, vch_sb[mj], False)):
                    r0 = tix * 128
                    r1 = (4 + tix) * 128
                    c = _r if cast else (lambda ap: ap)
                    nc.sync.dma_start(out=c(dst[:, 10:28]), in_=c(ag_out[r0 : r0 + 128, 0:18]))
                    nc.sync.dma_start(out=c(dst[:, 28:46]), in_=c(ag_out[r1 : r1 + 128, 0:18]))
                    nc.sync.dma_start(out=c(dst[:, 46:78]), in_=c(ag_out[r0 : r0 + 128, 18:50]))
                    nc.sync.dma_start(out=c(dst[:, 78:110]), in_=c(ag_out[r1 : r1 + 128, 18:50]))
            with tc.tile_pool(name="ps_tr", bufs=2, space="PSUM") as ps_tr:
                for mj in range(2):
                    for dst, cast in ((key_sb[mj], True), (vch_sb[mj], False)):
                        c = _r if cast else (lambda ap: ap)
                        # p3 grid from p6: p6 idx (within cols 10:46) = 12I+6di+2J+dj
                        nc.vector.tensor_reduce(
                            c(dst[:, 1:10].rearrange("p (a b) -> p a b", a=3)),
                            dst[:, 10:46].rearrange(
                                "p (I di J dj) -> p I J di dj", I=3, di=2, J=3
                            ),
                            axis=mybir.AxisListType.XY,
                            op=ALU.add,
                        )
                        nc.vector.tensor_reduce(
                            c(dst[:, 0:1]),
                            dst[:, 10:46],
                            axis=mybir.AxisListType.X,
                            op=ALU.add,
                        )
                    # key: sums -> means, fold in the 1/sqrt(ck) score scale
                    nc.vector.tensor_mul(_r(key_sb[mj]), key_sb[mj], kcr_sb)
                    # value: transpose to [s, cv], scale by 1/count
                    pt = ps_tr.tile([S, 128], F32, tag="ptr", name="ptr")
                    nc.tensor.transpose(pt, vch_sb[mj], ident_sb)
                    nc.scalar.activation(
                        _r(value_sb[:, mj * 128 : (mj + 1) * 128]),
                        pt,
                        AF.Copy,
                        scale=vcr_sb,
                    )
            nc.vector.tensor_add(_r(value_sb), value_sb, bv_sb)

            # ================= Phase C: attention + output conv ============
            with (
                tc.tile_pool(name="ps_sc", bufs=2, space="PSUM") as ps_sc,
                tc.tile_pool(name="ps_cs", bufs=2, space="PSUM") as ps_cs,
                tc.tile_pool(name="ps_ag", bufs=2, space="PSUM") as ps_ag,
                tc.tile_pool(name="ps_out", bufs=2, space="PSUM") as ps_out,
                tc.tile_pool(name="attn", bufs=2) as sb_attn,
                tc.tile_pool(name="aggp", bufs=2) as sb_agg,
                tc.tile_pool(name="outp", bufs=3) as sb_out,
            ):
                offs = list(range(0, NPIX, PIXT))
                for off in offs:
                    N = min(PIXT, NPIX - off)
                    psc = ps_sc.tile([S, PIXT], F32, tag="sc", name="sc")[:, :N]
                    for mj in range(2):
                        nc.tensor.matmul(
                            psc,
                            _r(key_sb[mj]),
                            _r(kq_sb[mj][:, off : off + N]),
                            start=(mj == 0),
                            stop=(mj == 1),
                        )
                    expt = sb_attn.tile([S, PIXT], F32, tag="exp", name="exp")[:, :N]
                    nc.scalar.activation(_r(expt), psc, AF.Exp)
                    pcs = ps_cs.tile([S, PIXT], F32, tag="cs", name="cs")[:, :N]
                    nc.tensor.matmul(pcs, _r(ones_sb), _r(expt), start=True, stop=True)
                    lncs = sb_attn.tile([S, PIXT], F32, tag="lncs", name="lncs")[:, :N]
                    nc.scalar.activation(lncs, pcs, AF.Ln)
                    rb = sb_attn.tile([S, PIXT], F32, tag="rb", name="rb")[:, :N]
                    nc.scalar.activation(rb, lncs, AF.Exp, scale=-1.0)
                    attn = sb_attn.tile([S, PIXT], F32, tag="attn", name="attn")[:, :N]
                    nc.vector.tensor_mul(_r(attn), expt, rb)
                    aggt = []
                    for mj in range(2):
                        pag = ps_ag.tile([128, PIXT], F32, tag="ag", name="ag")[:, :N]
                        nc.tensor.matmul(
                            pag,
                            _r(value_sb[:, mj * 128 : (mj + 1) * 128]),
                            _r(attn),
                            start=True,
                            stop=True,
                        )
                        at = sb_agg.tile([128, PIXT], F32, tag=f"agg{mj}", name=f"agg{mj}")[:, :N]
                        nc.scalar.activation(_r(at), pag, AF.Copy)
                        aggt.append(at)
                    for co in range(4):
                        po = ps_out.tile([128, PIXT], F32, tag="out", name="out")[:, :N]
                        for mj in range(2):
                            nc.tensor.matmul(
                                po,
                                _r(wo_sb[mj][:, co * 128 : (co + 1) * 128]),
                                _r(aggt[mj]),
                                start=(mj == 0),
                                stop=(mj == 1),
                            )
                        ot = sb_out.tile([128, PIXT], F32, tag="ot", name="ot")[:, :N]
                        if co % 2 == 0:
                            nc.scalar.activation(ot, po, AF.Identity, bias=bo_sb[co])
                        else:
                            nc.vector.tensor_scalar_add(ot, po, bo_sb[co])
                        nc.sync.dma_start(
                            out=out_d[co * 128 : (co + 1) * 128, off : off + N], in_=ot
                        )
    _split_excess_waits(nc)
    return nc


_CACHE = {}


def _get_nc():
    if "nc" not in _CACHE:
        _CACHE["nc"] = build_nc()
    return _CACHE["nc"]


def kernel(x, Wk, bk, gamma, beta, mean, var, Wv, bv, Wo, bo):
    from concourse.bass_utils import run_bass_kernel_spmd

    x = np.asarray(x, np.float32)
    Wk = np.asarray(Wk, np.float32)
    bk = np.asarray(bk, np.float32)
    gamma = np.asarray(gamma, np.float32)
    beta = np.asarray(beta, np.float32)
    mean = np.asarray(mean, np.float32)
    var = np.asarray(var, np.float32)
    Wv = np.asarray(Wv, np.float32)
    bv = np.asarray(bv, np.float32)
    Wo = np.asarray(Wo, np.float32)
    bo = np.asarray(bo, np.float32)

    inv = gamma / np.sqrt(var + EPS)
    shift = beta - mean * inv
    # conv-key bias bk feeds the BN shift: BN(relu input) of (Wk x + bk)
    # reference: kq = conv+bk, then kq*inv + (beta - mean*inv).  bk folds in:
    shift = shift + bk * inv

    counts = np.concatenate(
        [
            np.full(1, (H * W), np.float32),
            np.full(9, (H // 3) * (W // 3), np.float32),
            np.full(36, (H // 6) * (W // 6), np.float32),
            np.full(64, (H // 8) * (W // 8), np.float32),
        ]
    )
    key_crecip = np.broadcast_to(
        (1.0 / counts)[None, :] * (CK**-0.5), (128, S)
    ).astype(np.float32).copy()
    val_crecip = (1.0 / counts)[:, None].astype(np.float32).copy()

    common = {
        "wkT": np.ascontiguousarray(Wk.T),
        "wvT": np.ascontiguousarray(Wv.T),
        "woT": np.ascontiguousarray(Wo.T),
        "bn_inv": inv[:, None].copy(),
        "bn_shift": shift[:, None].copy(),
        "bo": bo[:, None].copy(),
        "bv_bcast": np.broadcast_to(bv[None, :], (S, CV)).astype(np.float32).copy(),
        "key_crecip": key_crecip,
        "val_crecip": val_crecip,
        "ones_mat": np.ones((S, S), np.float32),
        "ident": np.eye(128, dtype=np.float32),
    }
    in_maps = []
    for c in range(N_CORES):
        b, h = c // 2, c % 2
        m = dict(common)
        m["x_sh"] = np.ascontiguousarray(x[b, :, h * HL : (h + 1) * HL, :])
        in_maps.append(m)

    nc = _get_nc()
    _CACHE["last_in_maps"] = in_maps
    res = run_bass_kernel_spmd(nc, in_maps, core_ids=list(range(N_CORES)))
    out = np.empty((B, COUT, H, W), np.float32)
    for c in range(N_CORES):
        b, h = c // 2, c % 2
        out[b, :, h * HL : (h + 1) * HL, :] = res.results[c]["out"].reshape(
            COUT, HL, W
        )
    return out


# revision 13
# speedup vs baseline: 1.2046x; 1.2046x over previous
"""APNB block (sparse pyramid attention) distributed over 8 TRN2 NeuronCores.

Sharding: core c = 2*b + h handles batch item b (of 4) and row-half h (of 2,
60 rows each).  All three 1x1 convs and the attention are data-parallel over
the 7200 local pixels.  The only cross-core data are the pyramid-pooled
key/value sums: p6 (20x20) and p8 (15x15) pooling blocks split cleanly at the
row-60 boundary, so each pair of cores AllGathers its half-grids (p1/p3 are
derived from the p6 grid afterwards).

Phase order hides collective latency: [kq convs + kq pooling] -> AllGather#1
(kq grids) -> [v convs + v pooling] (covers AG1) -> AllGather#2 (v grids) ->
key fixup + early attention tiles (cover AG2) -> rest of attention.

Layouts (per core):
  kq  [ck=256, n=7200]   channel-major bf16, SBUF-resident (query & key src)
  scoresT [s=110, n]     pixels on the free axis
  value   [s=110, cv]    region-major (PE-transposed once after AG2)
  agg     [cv, n], out [co, n]  -> contiguous output DMA
Matmul operands are bf16 (fp32 PSUM accumulation); softmax reciprocal runs as
Exp(-Ln(x)) on the Scalar engine.
"""

import sys

sys.path.insert(0, "/opt/trn_rl_repo")

import numpy as np

import concourse.bass as bass
import concourse.mybir as mybir
import concourse.tile as tile
from concourse.vector_clock import ScopedClock

F32 = mybir.dt.float32
BF16 = mybir.dt.bfloat16
AF = mybir.ActivationFunctionType
ALU = mybir.AluOpType

N_CORES = 8
B, CIN, H, W = 4, 512, 120, 120
CK, CV, COUT = 256, 256, 512
HL = H // 2          # 60 local rows per core
NPIX = HL * W        # 7200 local pixels
S = 110              # 1 + 9 + 36 + 64 pooled regions
RG = 4               # rows per conv group
NGRP = HL // RG      # 15 conv groups
PIXT = 512           # attention pixel tile
EPS = 1e-5
# per-half pooled partial grid: p6 3x6=18 + p8 4x8=32 regions
HALF_GRID = 50


class SplitDrainTC(tile.TileContext):
    """TileContext whose kernel-tail drain splits its semaphore waits into
    one wait instruction per semaphore (walrus rejects multi-wait
    instructions, and the tail drain otherwise aggregates every live proc)."""

    def _drain_and_barrier(self, tick_clock, wait_clock):
        nc = self.nc
        nc.sync.drain()
        probe = mybir.InstNoOp(
            name="wait-probe", ins=[], outs=[], engine=mybir.EngineType.SP
        )
        wait_clock.add_sem_waits(probe, ScopedClock({None: tick_clock.global_clock}))
        waits = list(probe.sync_info.on_wait or []) if probe.sync_info else []
        name2handle = {
            getattr(h, "name", None): h for h in wait_clock.sems.allocated().values()
        }
        for w in waits:
            h = name2handle.get(w.ant_name)
            assert h is not None, f"no sem handle for {w.ant_name}"
            nc.sync.wait_ge(h, w.wait_value)
        nc.all_engine_barrier()
        popped = nc._tile_sem_poison_stack.pop()
        assert popped is self._sem_poison
        nc.clear_and_free_semaphores(list(self.sems.allocated().values()))
        nc.all_engine_barrier()


def _split_excess_waits(nc):
    """Walrus codegen rejects instructions with more than one sync wait
    (strictest struct: the fused LDWEIGHTS+MATMUL).  Move the excess onto
    EventSemaphore instructions just before the owner on the same engine
    queue, which preserves ordering semantics exactly."""
    for bb in nc.main_func.blocks:
        il = list(bb.instructions)
        out = []
        changed = False
        for ins in il:
            si = ins.sync_info
            waits = list(si.on_wait) if si is not None and si.on_wait else []
            if len(waits) > 1:
                changed = True
                pre, keep = waits[:-1], waits[-1:]
                for j, w in enumerate(pre):
                    ev = mybir.InstEventSemaphore(
                        name=f"wsplit-{ins.name}-{j}",
                        ins=[],
                        outs=[],
                        engine=ins.engine,
                        sync_info=mybir.SyncInfo(on_wait=[w], on_update=[]),
                    )
                    nc.register_instruction(ev)
                    out.append(ev)
                ins.sync_info = mybir.SyncInfo(
                    on_wait=keep, on_update=list(si.on_update or [])
                )
            out.append(ins)
        if changed:
            bb.instructions = out


def build_nc():
    nc = bass.Bass(num_devices=N_CORES)

    x_sh = nc.declare_dram_parameter("x_sh", [CIN, HL, W], BF16, isOutput=False)
    wkT = nc.declare_dram_parameter("wkT", [CIN, CK], BF16, isOutput=False)
    wvT = nc.declare_dram_parameter("wvT", [CIN, CV], BF16, isOutput=False)
    woT = nc.declare_dram_parameter("woT", [CV, COUT], BF16, isOutput=False)
    ones_mat = nc.declare_dram_parameter("ones_mat", [S, S], BF16, isOutput=False)
    ident = nc.declare_dram_parameter("ident", [128, 128], F32, isOutput=False)
    bn_inv = nc.declare_dram_parameter("bn_inv", [CK, 1], F32, isOutput=False)
    bn_shift = nc.declare_dram_parameter("bn_shift", [CK, 1], F32, isOutput=False)
    bo = nc.declare_dram_parameter("bo", [COUT, 1], F32, isOutput=False)
    bv_bcast = nc.declare_dram_parameter("bv_bcast", [S, CV], F32, isOutput=False)
    key_crecip = nc.declare_dram_parameter("key_crecip", [128, S], F32, isOutput=False)
    val_crecip = nc.declare_dram_parameter("val_crecip", [S, 1], F32, isOutput=False)
    out_d = nc.declare_dram_parameter("out", [COUT, NPIX], F32, isOutput=True)

    with nc.allow_low_precision("bf16 matmul pipeline"), SplitDrainTC(nc) as tc:
        with (
            tc.tile_pool(name="persist", bufs=1) as persist,
            tc.tile_pool(name="dram", bufs=1, space="DRAM") as dram,
        ):
            # ---- constants to SBUF ----
            wk_sb, wv_sb = [], []
            for ki in range(4):
                t = persist.tile([128, CK], BF16, tag=f"wk{ki}", name=f"wk{ki}")
                nc.sync.dma_start(out=t, in_=wkT[ki * 128 : (ki + 1) * 128, :])
                wk_sb.append(t)
                t = persist.tile([128, CV], BF16, tag=f"wv{ki}", name=f"wv{ki}")
                nc.sync.dma_start(out=t, in_=wvT[ki * 128 : (ki + 1) * 128, :])
                wv_sb.append(t)
            wo_sb = []
            for mj in range(2):
                t = persist.tile([128, COUT], BF16, tag=f"wo{mj}", name=f"wo{mj}")
                nc.sync.dma_start(out=t, in_=woT[mj * 128 : (mj + 1) * 128, :])
                wo_sb.append(t)
            bn_inv_sb, bn_shift_sb = [], []
            for mj in range(2):
                t = persist.tile([128, 1], F32, tag=f"bninv{mj}", name=f"bninv{mj}")
                nc.sync.dma_start(out=t, in_=bn_inv[mj * 128 : (mj + 1) * 128, :])
                bn_inv_sb.append(t)
                t = persist.tile([128, 1], F32, tag=f"bnsh{mj}", name=f"bnsh{mj}")
                nc.sync.dma_start(out=t, in_=bn_shift[mj * 128 : (mj + 1) * 128, :])
                bn_shift_sb.append(t)
            bo_sb = []
            for co in range(4):
                t = persist.tile([128, 1], F32, tag=f"bo{co}", name=f"bo{co}")
                nc.sync.dma_start(out=t, in_=bo[co * 128 : (co + 1) * 128, :])
                bo_sb.append(t)
            bv_sb = persist.tile([S, CV], F32, tag="bv", name="bv")
            nc.sync.dma_start(out=bv_sb, in_=bv_bcast[:, :])
            kcr_sb = persist.tile([128, S], F32, tag="kcr", name="kcr")
            nc.sync.dma_start(out=kcr_sb, in_=key_crecip[:, :])
            vcr_sb = persist.tile([S, 1], F32, tag="vcr", name="vcr")
            nc.sync.dma_start(out=vcr_sb, in_=val_crecip[:, :])
            ones_sb = persist.tile([S, S], BF16, tag="ones", name="ones")
            nc.sync.dma_start(out=ones_sb, in_=ones_mat[:, :])
            ident_sb = persist.tile([128, 128], F32, tag="ident", name="ident")
            nc.sync.dma_start(out=ident_sb, in_=ident[:, :])

            # ---- persistent working buffers ----
            kq_sb = [
                persist.tile([128, NPIX], BF16, tag=f"kq{mj}", name=f"kq{mj}")
                for mj in range(2)
            ]
            kq_rs5 = [
                persist.tile([128, HL, 24], BF16, tag=f"kqrs{mj}", name=f"kqrs{mj}")
                for mj in range(2)
            ]
            v_rs5 = [
                persist.tile([128, HL, 24], BF16, tag=f"vrs{mj}", name=f"vrs{mj}")
                for mj in range(2)
            ]
            kqp_sb = [
                persist.tile([128, HALF_GRID], F32, tag=f"kqp{mj}", name=f"kqp{mj}")
                for mj in range(2)
            ]
            vp_sb = [
                persist.tile([128, HALF_GRID], F32, tag=f"vp{mj}", name=f"vp{mj}")
                for mj in range(2)
            ]
            key_raw = [
                persist.tile([128, S], F32, tag=f"keyr{mj}", name=f"keyr{mj}")
                for mj in range(2)
            ]
            key_sb = [
                persist.tile([128, S], BF16, tag=f"key{mj}", name=f"key{mj}")
                for mj in range(2)
            ]
            vch_sb = [
                persist.tile([128, S], F32, tag=f"vch{mj}", name=f"vch{mj}")
                for mj in range(2)
            ]
            value_f = persist.tile([S, CV], F32, tag="valuef", name="valuef")
            value_sb = persist.tile([S, CV], BF16, tag="value", name="value")

            ag1_in = dram.tile([2 * 128, HALF_GRID], F32)
            ag1_out = dram.tile([4 * 128, HALF_GRID], F32)
            ag2_in = dram.tile([2 * 128, HALF_GRID], F32)
            ag2_out = dram.tile([4 * 128, HALF_GRID], F32)

            def combines(src, dst, pool):
                """rs5 [128, 60, 24] (5-col sums) -> p6 half grid (dst[:,0:18])
                and p8 half grid (dst[:,18:50])."""
                for mj in range(2):
                    t6 = pool.tile([128, HL, 6], BF16, tag="t6", name="t6")
                    nc.vector.tensor_reduce(
                        t6,
                        src[mj].rearrange("p r (j f) -> p r j f", f=4),
                        axis=mybir.AxisListType.X,
                        op=ALU.add,
                    )
                    nc.vector.tensor_reduce(
                        dst[mj][:, 0:18].rearrange("p (a b) -> p a b", a=3),
                        t6.rearrange("p (rb rr) j -> p rb j rr", rr=20),
                        axis=mybir.AxisListType.X,
                        op=ALU.add,
                    )
                    t8 = pool.tile([128, HL, 8], BF16, tag="t8", name="t8")
                    nc.vector.tensor_reduce(
                        t8,
                        src[mj].rearrange("p r (j f) -> p r j f", f=3),
                        axis=mybir.AxisListType.X,
                        op=ALU.add,
                    )
                    nc.vector.tensor_reduce(
                        dst[mj][:, 18:50].rearrange("p (a b) -> p a b", a=4),
                        t8.rearrange("p (rb rr) j -> p rb j rr", rr=15),
                        axis=mybir.AxisListType.X,
                        op=ALU.add,
                    )

            def ag_assemble(ag_out_t, dsts):
                """Place the gathered half grids into a [128, 110] region grid
                (cols: 0 p1 | 1:10 p3 | 10:46 p6 | 46:110 p8), then derive the
                p3/p1 sums from the p6 grid."""
                for mj in range(2):
                    dst = dsts[mj]
                    r0 = mj * 128
                    r1 = (2 + mj) * 128
                    nc.sync.dma_start(
                        out=dst[:, 10:28], in_=ag_out_t[r0 : r0 + 128, 0:18]
                    )
                    nc.sync.dma_start(
                        out=dst[:, 28:46], in_=ag_out_t[r1 : r1 + 128, 0:18]
                    )
                    nc.sync.dma_start(
                        out=dst[:, 46:78], in_=ag_out_t[r0 : r0 + 128, 18:50]
                    )
                    nc.sync.dma_start(
                        out=dst[:, 78:110], in_=ag_out_t[r1 : r1 + 128, 18:50]
                    )
                    nc.vector.tensor_reduce(
                        dst[:, 1:10].rearrange("p (a b) -> p a b", a=3),
                        dst[:, 10:46].rearrange(
                            "p (I di J dj) -> p I J di dj", I=3, di=2, J=3
                        ),
                        axis=mybir.AxisListType.XY,
                        op=ALU.add,
                    )
                    nc.vector.tensor_reduce(
                        dst[:, 0:1],
                        dst[:, 10:46],
                        axis=mybir.AxisListType.X,
                        op=ALU.add,
                    )

            rg = [[0, 1], [2, 3], [4, 5], [6, 7]]

            # ============ Phase A1: kq convs + pooling, then AG1 ============
            with (
                tc.tile_pool(name="xin", bufs=3) as xin,
                tc.tile_pool(name="comb", bufs=2) as comb,
                tc.tile_pool(name="ps_kq", bufs=2, space="PSUM") as ps_kq,
                tc.tile_pool(name="ps_v", bufs=2, space="PSUM") as ps_v,
            ):

                def load_x(g):
                    xt = xin.tile([128, 4, RG * W], BF16, tag="x", name="x")
                    nc.sync.dma_start(
                        out=xt.rearrange("p k (a b) -> p k a b", a=RG),
                        in_=bass.AP(
                            tensor=x_sh[:, :, :].tensor,
                            offset=g * RG * W,
                            ap=[
                                [HL * W, 128],
                                [128 * HL * W, 4],
                                [W, RG],
                                [1, W],
                            ],
                        ),
                    )
                    return xt

                for g in range(NGRP):
                    xt = load_x(g)
                    sl = slice(g * RG * W, (g + 1) * RG * W)
                    for mj in range(2):
                        pk = ps_kq.tile([128, RG * W], F32, tag="pkq", name="pkq")
                        for ki in range(4):
                            nc.tensor.matmul(
                                pk,
                                wk_sb[ki][:, mj * 128 : (mj + 1) * 128],
                                xt[:, ki, :],
                                start=(ki == 0),
                                stop=(ki == 3),
                            )
                        nc.scalar.activation(
                            kq_sb[mj][:, sl],
                            pk,
                            AF.Relu,
                            bias=bn_shift_sb[mj],
                            scale=bn_inv_sb[mj],
                        )
                        nc.vector.tensor_reduce(
                            kq_rs5[mj][:, g * RG : (g + 1) * RG, :],
                            kq_sb[mj][:, sl].rearrange(
                                "p (r c f) -> p r c f", r=RG, c=24
                            ),
                            axis=mybir.AxisListType.X,
                            op=ALU.add,
                        )
                combines(kq_rs5, kqp_sb, comb)
                for mj in range(2):
                    nc.sync.dma_start(
                        out=ag1_in[mj * 128 : (mj + 1) * 128, :], in_=kqp_sb[mj]
                    )
                nc.gpsimd.collective_compute(
                    "AllGather",
                    ALU.bypass,
                    replica_groups=rg,
                    ins=[ag1_in[:, :].opt()],
                    outs=[ag1_out[:, :].opt()],
                )

                # ============ Phase A2: v convs + pooling, then AG2 =========
                for g in range(NGRP):
                    xt = load_x(g)
                    for mj in range(2):
                        pv = ps_v.tile([128, RG * W], F32, tag="pv", name="pv")
                        for ki in range(4):
                            nc.tensor.matmul(
                                pv,
                                wv_sb[ki][:, mj * 128 : (mj + 1) * 128],
                                xt[:, ki, :],
                                start=(ki == 0),
                                stop=(ki == 3),
                            )
                        nc.vector.tensor_reduce(
                            v_rs5[mj][:, g * RG : (g + 1) * RG, :],
                            pv.rearrange("p (r c f) -> p r c f", r=RG, c=24),
                            axis=mybir.AxisListType.X,
                            op=ALU.add,
                        )
                combines(v_rs5, vp_sb, comb)
                for mj in range(2):
                    nc.sync.dma_start(
                        out=ag2_in[mj * 128 : (mj + 1) * 128, :], in_=vp_sb[mj]
                    )
                nc.gpsimd.collective_compute(
                    "AllGather",
                    ALU.bypass,
                    replica_groups=rg,
                    ins=[ag2_in[:, :].opt()],
                    outs=[ag2_out[:, :].opt()],
                )

            # ============ Phase B: key / value fixups =======================
            ag_assemble(ag1_out, key_raw)
            for mj in range(2):
                # sums -> means, fold in the 1/sqrt(ck) score scale; -> bf16
                nc.vector.tensor_mul(key_sb[mj], key_raw[mj], kcr_sb)
            ag_assemble(ag2_out, vch_sb)
            with tc.tile_pool(name="ps_tr", bufs=2, space="PSUM") as ps_tr:
                for mj in range(2):
                    pt = ps_tr.tile([S, 128], F32, tag="ptr", name="ptr")
                    nc.tensor.transpose(pt, vch_sb[mj], ident_sb)
                    nc.scalar.activation(
                        value_f[:, mj * 128 : (mj + 1) * 128],
                        pt,
                        AF.Copy,
                        scale=vcr_sb,
                    )
            nc.vector.tensor_add(value_sb, value_f, bv_sb)

            # ============ Phase C: attention + output conv ==================
            with (
                tc.tile_pool(name="ps_sc", bufs=2, space="PSUM") as ps_sc,
                tc.tile_pool(name="ps_cs", bufs=2, space="PSUM") as ps_cs,
                tc.tile_pool(name="ps_ag", bufs=2, space="PSUM") as ps_ag,
                tc.tile_pool(name="ps_out", bufs=2, space="PSUM") as ps_out,
                tc.tile_pool(name="attn", bufs=3) as sb_attn,
                tc.tile_pool(name="aggp", bufs=2) as sb_agg,
                tc.tile_pool(name="outp", bufs=3) as sb_out,
            ):
                for off in range(0, NPIX, PIXT):
                    N = min(PIXT, NPIX - off)
                    psc = ps_sc.tile([S, PIXT], F32, tag="sc", name="sc")[:, :N]
                    for mj in range(2):
                        nc.tensor.matmul(
                            psc,
                            key_sb[mj],
                            kq_sb[mj][:, off : off + N],
                            start=(mj == 0),
                            stop=(mj == 1),
                        )
                    expt = sb_attn.tile([S, PIXT], BF16, tag="exp", name="exp")[:, :N]
                    nc.scalar.activation(expt, psc, AF.Exp)
                    pcs = ps_cs.tile([S, PIXT], F32, tag="cs", name="cs")[:, :N]
                    nc.tensor.matmul(pcs, ones_sb, expt, start=True, stop=True)
                    lncs = sb_attn.tile([S, PIXT], F32, tag="lncs", name="lncs")[:, :N]
                    nc.scalar.activation(lncs, pcs, AF.Ln)
                    rb = sb_attn.tile([S, PIXT], BF16, tag="rb", name="rb")[:, :N]
                    nc.scalar.activation(rb, lncs, AF.Exp, scale=-1.0)
                    attn = sb_attn.tile([S, PIXT], BF16, tag="attn", name="attn")[
                        :, :N
                    ]
                    nc.vector.tensor_mul(attn, expt, rb)
                    aggt = []
                    for mj in range(2):
                        pag = ps_ag.tile([128, PIXT], F32, tag="ag", name="ag")[:, :N]
                        nc.tensor.matmul(
                            pag,
                            value_sb[:, mj * 128 : (mj + 1) * 128],
                            attn,
                            start=True,
                            stop=True,
                        )
                        at = sb_agg.tile(
                            [128, PIXT], BF16, tag=f"agg{mj}", name=f"agg{mj}"
                        )[:, :N]
                        nc.scalar.activation(at, pag, AF.Copy)
                        aggt.append(at)
                    ot = sb_out.tile([128, 4, PIXT], F32, tag="ot", name="ot")[
                        :, :, :N
                    ]
                    for co in range(4):
                        po = ps_out.tile([128, PIXT], F32, tag="out", name="po")[
                            :, :N
                        ]
                        for mj in range(2):
                            nc.tensor.matmul(
                                po,
                                wo_sb[mj][:, co * 128 : (co + 1) * 128],
                                aggt[mj],
                                start=(mj == 0),
                                stop=(mj == 1),
                            )
                        if co % 2 == 0:
                            nc.scalar.activation(
                                ot[:, co, :], po, AF.Identity, bias=bo_sb[co]
                            )
                        else:
                            nc.vector.tensor_scalar_add(ot[:, co, :], po, bo_sb[co])
                    nc.sync.dma_start(
                        out=bass.AP(
                            tensor=out_d[:, :].tensor,
                            offset=off,
                            ap=[[NPIX, 128], [128 * NPIX, 4], [1, N]],
                        ),
                        in_=ot,
                    )
    _split_excess_waits(nc)
    return nc


_CACHE = {}


def _get_nc():
    if "nc" not in _CACHE:
        _CACHE["nc"] = build_nc()
    return _CACHE["nc"]


def kernel(x, Wk, bk, gamma, beta, mean, var, Wv, bv, Wo, bo):
    import ml_dtypes

    from concourse.bass_utils import run_bass_kernel_spmd

    bf16 = ml_dtypes.bfloat16
    x = np.asarray(x, np.float32)
    Wk = np.asarray(Wk, np.float32)
    bk = np.asarray(bk, np.float32)
    gamma = np.asarray(gamma, np.float32)
    beta = np.asarray(beta, np.float32)
    mean = np.asarray(mean, np.float32)
    var = np.asarray(var, np.float32)
    Wv = np.asarray(Wv, np.float32)
    bv = np.asarray(bv, np.float32)
    Wo = np.asarray(Wo, np.float32)
    bo = np.asarray(bo, np.float32)

    inv = gamma / np.sqrt(var + EPS)
    # reference: kq = (Wk x + bk), then BN: kq*inv + (beta - mean*inv)
    shift = beta - mean * inv + bk * inv

    counts = np.concatenate(
        [
            np.full(1, H * W, np.float32),
            np.full(9, (H // 3) * (W // 3), np.float32),
            np.full(36, (H // 6) * (W // 6), np.float32),
            np.full(64, (H // 8) * (W // 8), np.float32),
        ]
    )
    key_crecip = (
        np.broadcast_to((1.0 / counts)[None, :] * (CK**-0.5), (128, S))
        .astype(np.float32)
        .copy()
    )
    val_crecip = (1.0 / counts)[:, None].astype(np.float32).copy()

    common = {
        "wkT": np.ascontiguousarray(Wk.T).astype(bf16),
        "wvT": np.ascontiguousarray(Wv.T).astype(bf16),
        "woT": np.ascontiguousarray(Wo.T).astype(bf16),
        "ones_mat": np.ones((S, S), bf16),
        "ident": np.eye(128, dtype=np.float32),
        "bn_inv": inv[:, None].copy(),
        "bn_shift": shift[:, None].copy(),
        "bo": bo[:, None].copy(),
        "bv_bcast": np.broadcast_to(bv[None, :], (S, CV)).astype(np.float32).copy(),
        "key_crecip": key_crecip,
        "val_crecip": val_crecip,
    }
    in_maps = []
    for c in range(N_CORES):
        b, h = c // 2, c % 2
        m = dict(common)
        m["x_sh"] = np.ascontiguousarray(x[b, :, h * HL : (h + 1) * HL, :]).astype(
            bf16
        )
        in_maps.append(m)

    nc = _get_nc()
    _CACHE["last_in_maps"] = in_maps
    res = run_bass_kernel_spmd(nc, in_maps, core_ids=list(range(N_CORES)))
    out = np.empty((B, COUT, H, W), np.float32)
    for c in range(N_CORES):
        b, h = c // 2, c % 2
        out[b, :, h * HL : (h + 1) * HL, :] = res.results[c]["out"].reshape(
            COUT, HL, W
        )
    return out


# revision 14
# speedup vs baseline: 1.3433x; 1.1152x over previous
"""APNB block (sparse pyramid attention) distributed over 8 TRN2 NeuronCores.

Sharding: core c = 2*b + h handles batch item b (of 4) and row-half h (of 2,
60 rows each).  All three 1x1 convs and the attention are data-parallel over
the 7200 local pixels.  The only cross-core data are the pyramid-pooled
key/value sums: p6 (20x20) and p8 (15x15) pooling blocks split cleanly at the
row-60 boundary, so each pair of cores AllGathers its half grids (p1/p3 are
derived from the p6 grid afterwards).

Structure (engine assignment in parens):
  A1: kq = relu(BN(Wk x)) convs (PE+ACT) + p6/p8 column sums (DVE); AllGather#1
  A2: v = Wv x convs (PE) + pooling from PSUM (DVE); AllGather#2
      -- A2's matmuls hide AG1; AG2 hides under the head of phase C.
  C:  per 512-pixel tile, software-pipelined:
        C1: scoresT = key^T kq (PE) -> exp (ACT) -> colsum via ones-matrix
            matmul (PE) -> 1/x as Exp(-Ln(x)) (ACT) -> attn = exp*recip (GpSimd)
        C2 (lags C1 by 5 tiles, covering AG2 + value fixup):
            out = WoV^T attn (PE) + bias evac (ACT/DVE) -> DMA
      where WoV[s, co] = value[s, :] @ Wo^T is precomputed once per core (the
      value matmul is folded into the output conv; Wo@bv folds into the bias).

All matmul operands are bf16 (fp32 PSUM accumulation).
"""

import sys

sys.path.insert(0, "/opt/trn_rl_repo")

import numpy as np

import concourse.bass as bass
import concourse.mybir as mybir
import concourse.tile as tile
from concourse.vector_clock import ScopedClock

F32 = mybir.dt.float32
BF16 = mybir.dt.bfloat16
AF = mybir.ActivationFunctionType
ALU = mybir.AluOpType

N_CORES = 8
B, CIN, H, W = 4, 512, 120, 120
CK, CV, COUT = 256, 256, 512
HL = H // 2          # 60 local rows per core
NPIX = HL * W        # 7200 local pixels
S = 110              # 1 + 9 + 36 + 64 pooled regions
RG = 4               # rows per conv group
NGRP = HL // RG      # 15 conv groups
PIXT = 512           # attention pixel tile
EPS = 1e-5
HALF_GRID = 50       # p6 3x6=18 + p8 4x8=32 per-half regions
LAG = 5              # C2 tiles lag C1 tiles by this much


class SplitDrainTC(tile.TileContext):
    """TileContext whose kernel-tail drain splits its semaphore waits into
    one wait instruction per semaphore (walrus rejects multi-wait
    instructions, and the tail drain otherwise aggregates every live proc)."""

    def _drain_and_barrier(self, tick_clock, wait_clock):
        nc = self.nc
        nc.sync.drain()
        probe = mybir.InstNoOp(
            name="wait-probe", ins=[], outs=[], engine=mybir.EngineType.SP
        )
        wait_clock.add_sem_waits(probe, ScopedClock({None: tick_clock.global_clock}))
        waits = list(probe.sync_info.on_wait or []) if probe.sync_info else []
        name2handle = {
            getattr(h, "name", None): h for h in wait_clock.sems.allocated().values()
        }
        for w in waits:
            h = name2handle.get(w.ant_name)
            assert h is not None, f"no sem handle for {w.ant_name}"
            nc.sync.wait_ge(h, w.wait_value)
        nc.all_engine_barrier()
        popped = nc._tile_sem_poison_stack.pop()
        assert popped is self._sem_poison
        nc.clear_and_free_semaphores(list(self.sems.allocated().values()))
        nc.all_engine_barrier()


def _split_excess_waits(nc):
    """Walrus codegen rejects instructions with more than one sync wait
    (strictest struct: the fused LDWEIGHTS+MATMUL).  Move the excess onto
    EventSemaphore instructions just before the owner on the same engine
    queue, which preserves ordering semantics exactly."""
    for bb in nc.main_func.blocks:
        il = list(bb.instructions)
        out = []
        changed = False
        for ins in il:
            si = ins.sync_info
            waits = list(si.on_wait) if si is not None and si.on_wait else []
            if len(waits) > 1:
                changed = True
                pre, keep = waits[:-1], waits[-1:]
                for j, w in enumerate(pre):
                    ev = mybir.InstEventSemaphore(
                        name=f"wsplit-{ins.name}-{j}",
                        ins=[],
                        outs=[],
                        engine=ins.engine,
                        sync_info=mybir.SyncInfo(on_wait=[w], on_update=[]),
                    )
                    nc.register_instruction(ev)
                    out.append(ev)
                ins.sync_info = mybir.SyncInfo(
                    on_wait=keep, on_update=list(si.on_update or [])
                )
            out.append(ins)
        if changed:
            bb.instructions = out


def build_nc():
    nc = bass.Bass(num_devices=N_CORES)

    x_sh = nc.declare_dram_parameter("x_sh", [CIN, HL, W], BF16, isOutput=False)
    wkT = nc.declare_dram_parameter("wkT", [CIN, CK], BF16, isOutput=False)
    wvT = nc.declare_dram_parameter("wvT", [CIN, CV], BF16, isOutput=False)
    woT = nc.declare_dram_parameter("woT", [CV, COUT], BF16, isOutput=False)
    ones_mat = nc.declare_dram_parameter("ones_mat", [S, S], BF16, isOutput=False)
    bn_inv = nc.declare_dram_parameter("bn_inv", [CK, 1], F32, isOutput=False)
    bn_shift = nc.declare_dram_parameter("bn_shift", [CK, 1], F32, isOutput=False)
    bo_eff = nc.declare_dram_parameter("bo_eff", [COUT, 1], F32, isOutput=False)
    key_crecip = nc.declare_dram_parameter("key_crecip", [128, S], F32, isOutput=False)
    val_crecip = nc.declare_dram_parameter("val_crecip", [128, S], F32, isOutput=False)
    out_d = nc.declare_dram_parameter("out", [COUT, NPIX], F32, isOutput=True)

    with nc.allow_low_precision("bf16 matmul pipeline"), SplitDrainTC(nc) as tc:
        with (
            tc.tile_pool(name="persist", bufs=1) as persist,
            tc.tile_pool(name="dram", bufs=1, space="DRAM") as dram,
        ):
            # ---- constants to SBUF (on the ACT DMA queue: keeps the SP
            # queue free for the first x tiles) ----
            wk_sb, wv_sb = [], []
            for ki in range(4):
                t = persist.tile([128, CK], BF16, tag=f"wk{ki}", name=f"wk{ki}")
                nc.scalar.dma_start(out=t, in_=wkT[ki * 128 : (ki + 1) * 128, :])
                wk_sb.append(t)
                t = persist.tile([128, CV], BF16, tag=f"wv{ki}", name=f"wv{ki}")
                nc.scalar.dma_start(out=t, in_=wvT[ki * 128 : (ki + 1) * 128, :])
                wv_sb.append(t)
            wo_sb = []
            for mj in range(2):
                t = persist.tile([128, COUT], BF16, tag=f"wo{mj}", name=f"wo{mj}")
                nc.scalar.dma_start(out=t, in_=woT[mj * 128 : (mj + 1) * 128, :])
                wo_sb.append(t)
            bn_inv_sb, bn_shift_sb = [], []
            for mj in range(2):
                t = persist.tile([128, 1], F32, tag=f"bninv{mj}", name=f"bninv{mj}")
                nc.scalar.dma_start(out=t, in_=bn_inv[mj * 128 : (mj + 1) * 128, :])
                bn_inv_sb.append(t)
                t = persist.tile([128, 1], F32, tag=f"bnsh{mj}", name=f"bnsh{mj}")
                nc.scalar.dma_start(out=t, in_=bn_shift[mj * 128 : (mj + 1) * 128, :])
                bn_shift_sb.append(t)
            bo_sb = []
            for co in range(4):
                t = persist.tile([128, 1], F32, tag=f"bo{co}", name=f"bo{co}")
                nc.scalar.dma_start(out=t, in_=bo_eff[co * 128 : (co + 1) * 128, :])
                bo_sb.append(t)
            kcr_sb = persist.tile([128, S], F32, tag="kcr", name="kcr")
            nc.scalar.dma_start(out=kcr_sb, in_=key_crecip[:, :])
            vcr_sb = persist.tile([128, S], F32, tag="vcr", name="vcr")
            nc.scalar.dma_start(out=vcr_sb, in_=val_crecip[:, :])
            ones_sb = persist.tile([S, S], BF16, tag="ones", name="ones")
            nc.scalar.dma_start(out=ones_sb, in_=ones_mat[:, :])

            # ---- persistent working buffers ----
            kq_sb = [
                persist.tile([128, NPIX], BF16, tag=f"kq{mj}", name=f"kq{mj}")
                for mj in range(2)
            ]
            kq_rs6 = [
                persist.tile([128, HL, 6], BF16, tag=f"kqr6{mj}", name=f"kqr6{mj}")
                for mj in range(2)
            ]
            kq_rs8 = [
                persist.tile([128, HL, 8], BF16, tag=f"kqr8{mj}", name=f"kqr8{mj}")
                for mj in range(2)
            ]
            v_rs6 = [
                persist.tile([128, HL, 6], BF16, tag=f"vr6{mj}", name=f"vr6{mj}")
                for mj in range(2)
            ]
            v_rs8 = [
                persist.tile([128, HL, 8], BF16, tag=f"vr8{mj}", name=f"vr8{mj}")
                for mj in range(2)
            ]
            kqp_sb = [
                persist.tile([128, HALF_GRID], F32, tag=f"kqp{mj}", name=f"kqp{mj}")
                for mj in range(2)
            ]
            vp_sb = [
                persist.tile([128, HALF_GRID], F32, tag=f"vp{mj}", name=f"vp{mj}")
                for mj in range(2)
            ]
            key_raw = [
                persist.tile([128, S], F32, tag=f"keyr{mj}", name=f"keyr{mj}")
                for mj in range(2)
            ]
            key_sb = [
                persist.tile([128, S], BF16, tag=f"key{mj}", name=f"key{mj}")
                for mj in range(2)
            ]
            vch_sb = [
                persist.tile([128, S], F32, tag=f"vch{mj}", name=f"vch{mj}")
                for mj in range(2)
            ]
            vchs_sb = [
                persist.tile([128, S], BF16, tag=f"vchs{mj}", name=f"vchs{mj}")
                for mj in range(2)
            ]
            wovT_sb = persist.tile([S, COUT], BF16, tag="wovT", name="wovT")

            ag1_in = dram.tile([2 * 128, HALF_GRID], F32)
            ag1_out = dram.tile([4 * 128, HALF_GRID], F32)
            ag2_in = dram.tile([2 * 128, HALF_GRID], F32)
            ag2_out = dram.tile([4 * 128, HALF_GRID], F32)

            def pool_cols(src_ap, rs6, rs8, g):
                """Column-block sums of a 4-row conv group into the running
                per-row p6/p8 column-sum buffers."""
                nc.vector.tensor_reduce(
                    rs6[:, g * RG : (g + 1) * RG, :],
                    src_ap.rearrange("p (r j f) -> p r j f", r=RG, j=6),
                    axis=mybir.AxisListType.X,
                    op=ALU.add,
                )
                nc.vector.tensor_reduce(
                    rs8[:, g * RG : (g + 1) * RG, :],
                    src_ap.rearrange("p (r j f) -> p r j f", r=RG, j=8),
                    axis=mybir.AxisListType.X,
                    op=ALU.add,
                )

            def row_combine(rs6, rs8, dst):
                """Row-block sums: per-row column sums -> p6/p8 half grids."""
                for mj in range(2):
                    nc.vector.tensor_reduce(
                        dst[mj][:, 0:18].rearrange("p (a b) -> p a b", a=3),
                        rs6[mj].rearrange("p (rb rr) j -> p rb j rr", rr=20),
                        axis=mybir.AxisListType.X,
                        op=ALU.add,
                    )
                    nc.vector.tensor_reduce(
                        dst[mj][:, 18:50].rearrange("p (a b) -> p a b", a=4),
                        rs8[mj].rearrange("p (rb rr) j -> p rb j rr", rr=15),
                        axis=mybir.AxisListType.X,
                        op=ALU.add,
                    )

            def ag_assemble(ag_out_t, dsts):
                """Gathered half grids -> [128, 110] region grid (cols: 0 p1 |
                1:10 p3 | 10:46 p6 | 46:110 p8) + derive p3/p1 from p6.
                DMAs ride the gpsimd queue (they wait on the collective)."""
                for mj in range(2):
                    dst = dsts[mj]
                    r0 = mj * 128
                    r1 = (2 + mj) * 128
                    nc.gpsimd.dma_start(
                        out=dst[:, 10:28], in_=ag_out_t[r0 : r0 + 128, 0:18]
                    )
                    nc.gpsimd.dma_start(
                        out=dst[:, 28:46], in_=ag_out_t[r1 : r1 + 128, 0:18]
                    )
                    nc.gpsimd.dma_start(
                        out=dst[:, 46:78], in_=ag_out_t[r0 : r0 + 128, 18:50]
                    )
                    nc.gpsimd.dma_start(
                        out=dst[:, 78:110], in_=ag_out_t[r1 : r1 + 128, 18:50]
                    )
                    nc.vector.tensor_reduce(
                        dst[:, 1:10].rearrange("p (a b) -> p a b", a=3),
                        dst[:, 10:46].rearrange(
                            "p (I di J dj) -> p I J di dj", I=3, di=2, J=3
                        ),
                        axis=mybir.AxisListType.XY,
                        op=ALU.add,
                    )
                    nc.vector.tensor_reduce(
                        dst[:, 0:1],
                        dst[:, 10:46],
                        axis=mybir.AxisListType.X,
                        op=ALU.add,
                    )

            rg = [[0, 1], [2, 3], [4, 5], [6, 7]]

            with (
                tc.tile_pool(name="xin", bufs=3) as xin,
                tc.tile_pool(name="ps_kq", bufs=2, space="PSUM") as ps_kq,
                tc.tile_pool(name="ps_v", bufs=2, space="PSUM") as ps_v,
            ):

                def load_x(g):
                    xt = xin.tile([128, 4, RG * W], BF16, tag="x", name="x")
                    nc.sync.dma_start(
                        out=xt.rearrange("p k (a b) -> p k a b", a=RG),
                        in_=bass.AP(
                            tensor=x_sh[:, :, :].tensor,
                            offset=g * RG * W,
                            ap=[
                                [HL * W, 128],
                                [128 * HL * W, 4],
                                [W, RG],
                                [1, W],
                            ],
                        ),
                    )
                    return xt

                # ============ Phase A1: kq convs + pooling, AG1 =============
                for g in range(NGRP):
                    xt = load_x(g)
                    sl = slice(g * RG * W, (g + 1) * RG * W)
                    for mj in range(2):
                        pk = ps_kq.tile([128, RG * W], F32, tag="pkq", name="pkq")
                        for ki in range(4):
                            nc.tensor.matmul(
                                pk,
                                wk_sb[ki][:, mj * 128 : (mj + 1) * 128],
                                xt[:, ki, :],
                                start=(ki == 0),
                                stop=(ki == 3),
                            )
                        nc.scalar.activation(
                            kq_sb[mj][:, sl],
                            pk,
                            AF.Relu,
                            bias=bn_shift_sb[mj],
                            scale=bn_inv_sb[mj],
                        )
                        pool_cols(kq_sb[mj][:, sl], kq_rs6[mj], kq_rs8[mj], g)
                row_combine(kq_rs6, kq_rs8, kqp_sb)
                for mj in range(2):
                    nc.gpsimd.dma_start(
                        out=ag1_in[mj * 128 : (mj + 1) * 128, :], in_=kqp_sb[mj]
                    )
                nc.gpsimd.collective_compute(
                    "AllGather",
                    ALU.bypass,
                    replica_groups=rg,
                    ins=[ag1_in[:, :].opt()],
                    outs=[ag1_out[:, :].opt()],
                )

                # ============ Phase A2: v convs + pooling, AG2 ==============
                for g in range(NGRP):
                    xt = load_x(g)
                    for mj in range(2):
                        pv = ps_v.tile([128, RG * W], F32, tag="pv", name="pv")
                        for ki in range(4):
                            nc.tensor.matmul(
                                pv,
                                wv_sb[ki][:, mj * 128 : (mj + 1) * 128],
                                xt[:, ki, :],
                                start=(ki == 0),
                                stop=(ki == 3),
                            )
                        pool_cols(pv[:, :], v_rs6[mj], v_rs8[mj], g)
                row_combine(v_rs6, v_rs8, vp_sb)
                for mj in range(2):
                    nc.gpsimd.dma_start(
                        out=ag2_in[mj * 128 : (mj + 1) * 128, :], in_=vp_sb[mj]
                    )
                nc.gpsimd.collective_compute(
                    "AllGather",
                    ALU.bypass,
                    replica_groups=rg,
                    ins=[ag2_in[:, :].opt()],
                    outs=[ag2_out[:, :].opt()],
                )

            # ---- key fixup (AG1 completed during phase A2) ----
            ag_assemble(ag1_out, key_raw)
            for mj in range(2):
                # sums -> means with the 1/sqrt(ck) score scale folded in
                nc.vector.tensor_mul(key_sb[mj], key_raw[mj], kcr_sb)

            # ============ Phase C: attention, software-pipelined ============
            with (
                tc.tile_pool(name="ps_sc", bufs=2, space="PSUM") as ps_sc,
                tc.tile_pool(name="ps_cs", bufs=2, space="PSUM") as ps_cs,
                tc.tile_pool(name="ps_wov", bufs=1, space="PSUM") as ps_wov,
                tc.tile_pool(name="ps_out", bufs=3, space="PSUM") as ps_out,
                tc.tile_pool(name="attn", bufs=3) as sb_c1,
                tc.tile_pool(name="attn_keep", bufs=LAG + 2) as sb_attn,
                tc.tile_pool(name="outp", bufs=3) as sb_out,
            ):
                offs = list(range(0, NPIX, PIXT))
                attn_tiles = {}

                def emit_c1(t):
                    off = offs[t]
                    N = min(PIXT, NPIX - off)
                    psc = ps_sc.tile([S, PIXT], F32, tag="sc", name="sc")[:, :N]
                    for mj in range(2):
                        nc.tensor.matmul(
                            psc,
                            key_sb[mj],
                            kq_sb[mj][:, off : off + N],
                            start=(mj == 0),
                            stop=(mj == 1),
                        )
                    expt = sb_c1.tile([S, PIXT], BF16, tag="exp", name="exp")[:, :N]
                    nc.scalar.activation(expt, psc, AF.Exp)
                    pcs = ps_cs.tile([S, PIXT], F32, tag="cs", name="cs")[:, :N]
                    nc.tensor.matmul(pcs, ones_sb, expt, start=True, stop=True)
                    lncs = sb_c1.tile([S, PIXT], F32, tag="lncs", name="lncs")[:, :N]
                    nc.scalar.activation(lncs, pcs, AF.Ln)
                    rb = sb_c1.tile([S, PIXT], BF16, tag="rb", name="rb")[:, :N]
                    nc.scalar.activation(rb, lncs, AF.Exp, scale=-1.0)
                    attn = sb_attn.tile([S, PIXT], BF16, tag="attn", name="attn")[
                        :, :N
                    ]
                    nc.gpsimd.tensor_mul(attn, expt, rb)
                    attn_tiles[t] = attn

                def emit_c2(t):
                    off = offs[t]
                    N = min(PIXT, NPIX - off)
                    attn = attn_tiles.pop(t)
                    ot = sb_out.tile([128, 4, PIXT], F32, tag="ot", name="ot")[
                        :, :, :N
                    ]
                    for co in range(4):
                        po = ps_out.tile([128, PIXT], F32, tag="out", name="po")[
                            :, :N
                        ]
                        nc.tensor.matmul(
                            po,
                            wovT_sb[:, co * 128 : (co + 1) * 128],
                            attn,
                            start=True,
                            stop=True,
                        )
                        if co % 2 == 0:
                            nc.scalar.activation(
                                ot[:, co, :], po, AF.Identity, bias=bo_sb[co]
                            )
                        else:
                            nc.vector.tensor_scalar_add(ot[:, co, :], po, bo_sb[co])
                    nc.sync.dma_start(
                        out=bass.AP(
                            tensor=out_d[:, :].tensor,
                            offset=off,
                            ap=[[NPIX, 128], [128 * NPIX, 4], [1, N]],
                        ),
                        in_=ot,
                    )

                for t in range(len(offs)):
                    emit_c1(t)
                    if t == LAG - 1:
                        # value fixup + WoV^T = (value/count) @ Wo^T (AG2 has
                        # landed under the first C1 tiles by now)
                        ag_assemble(ag2_out, vch_sb)
                        for mj in range(2):
                            nc.vector.tensor_mul(vchs_sb[mj], vch_sb[mj], vcr_sb)
                        pwov = ps_wov.tile([S, COUT], F32, tag="wov", name="pwov")
                        for mj in range(2):
                            nc.tensor.matmul(
                                pwov,
                                vchs_sb[mj],
                                wo_sb[mj],
                                start=(mj == 0),
                                stop=(mj == 1),
                            )
                        nc.scalar.activation(wovT_sb, pwov, AF.Copy)
                    if t >= LAG:
                        emit_c2(t - LAG)
                for t in range(len(offs) - LAG, len(offs)):
                    emit_c2(t)
    _split_excess_waits(nc)
    return nc


_CACHE = {}


def _get_nc():
    if "nc" not in _CACHE:
        _CACHE["nc"] = build_nc()
    return _CACHE["nc"]


def kernel(x, Wk, bk, gamma, beta, mean, var, Wv, bv, Wo, bo):
    import ml_dtypes

    from concourse.bass_utils import run_bass_kernel_spmd

    bf16 = ml_dtypes.bfloat16
    x = np.asarray(x, np.float32)
    Wk = np.asarray(Wk, np.float32)
    bk = np.asarray(bk, np.float32)
    gamma = np.asarray(gamma, np.float32)
    beta = np.asarray(beta, np.float32)
    mean = np.asarray(mean, np.float32)
    var = np.asarray(var, np.float32)
    Wv = np.asarray(Wv, np.float32)
    bv = np.asarray(bv, np.float32)
    Wo = np.asarray(Wo, np.float32)
    bo = np.asarray(bo, np.float32)

    inv = gamma / np.sqrt(var + EPS)
    # reference: kq = (Wk x + bk), then BN: kq*inv + (beta - mean*inv)
    shift = beta - mean * inv + bk * inv
    # value = pooled_mean + bv, and out = Wo @ value ... so Wo @ bv joins bias
    bo_eff = bo + Wo @ bv

    counts = np.concatenate(
        [
            np.full(1, H * W, np.float32),
            np.full(9, (H // 3) * (W // 3), np.float32),
            np.full(36, (H // 6) * (W // 6), np.float32),
            np.full(64, (H // 8) * (W // 8), np.float32),
        ]
    )
    key_crecip = (
        np.broadcast_to((1.0 / counts)[None, :] * (CK**-0.5), (128, S))
        .astype(np.float32)
        .copy()
    )
    val_crecip = (
        np.broadcast_to((1.0 / counts)[None, :], (128, S)).astype(np.float32).copy()
    )

    common = {
        "wkT": np.ascontiguousarray(Wk.T).astype(bf16),
        "wvT": np.ascontiguousarray(Wv.T).astype(bf16),
        "woT": np.ascontiguousarray(Wo.T).astype(bf16),
        "ones_mat": np.ones((S, S), bf16),
        "bn_inv": inv[:, None].copy(),
        "bn_shift": shift[:, None].copy(),
        "bo_eff": bo_eff[:, None].copy(),
        "key_crecip": key_crecip,
        "val_crecip": val_crecip,
    }
    in_maps = []
    for c in range(N_CORES):
        b, h = c // 2, c % 2
        m = dict(common)
        m["x_sh"] = np.ascontiguousarray(x[b, :, h * HL : (h + 1) * HL, :]).astype(
            bf16
        )
        in_maps.append(m)

    nc = _get_nc()
    _CACHE["last_in_maps"] = in_maps
    res = run_bass_kernel_spmd(nc, in_maps, core_ids=list(range(N_CORES)))
    out = np.empty((B, COUT, H, W), np.float32)
    for c in range(N_CORES):
        b, h = c // 2, c % 2
        out[b, :, h * HL : (h + 1) * HL, :] = res.results[c]["out"].reshape(
            COUT, HL, W
        )
    return out
